# revision 17
# baseline (speedup 1.0000x reference)
"""Trainium2 Bass kernel for nn_MultiHeadDynamics (v3, fp16 IO, pipelined).

Computation (per sample row x of state, s of signal):
    heads   = x.reshape(H, DH)                      # H=16, DH=256
    A_h     = U_h @ V_h + diag(d_h)                 # (DH, DH) per head
    lin     = heads @ A_h^T
    c       = heads - mean_dh(heads)
    drift   = lin + cs * c^3 + s
    out     = x + DT*(1+cp)*drift - (DT*cp/H) * sum_h(drift_h)

Folding:  beta = DT*(1+cp);  gp = DT*cp/(H*beta);  gam = cbrt(beta*cs)
    D'      = beta*drift = heads @ (beta*A)^T + (gam*c)^3 + beta*s
    out     = x + D' - gp * sum_h(D'_h)

v3 design:
  - IO in fp16 (host casts); halves HBM traffic. Pipeline error ~1e-3
    vs the 2e-2 gate.
  - PE: fp16 transposes; per-head matmuls vs (beta*A)^T + N=1 mean
    extractors; identity-stationary matmuls accumulate beta*s and c3
    into the same PSUM region, so PSUM holds the full drift.
    Each 256-wide PSUM region is a well-formed accumulation group
    closed with stop=True on its last member.
  - The PE stream has no DVE-dependent instructions (c3 joins drift in
    a DVE pass), so the PE runs the whole tile uninterrupted and stays
    at its max p-state.
  - DVE: ct = gam*(x-m) per head (m read from PSUM), c3 quarters,
    coupling tree, dd broadcast add, half the final out.
  - ACT: c2 = Square(ct) quarters + drift PSUM->SBUF copies + 2 hT
    copies.  GpSimd: t4 tree level + the other half of out.
"""

import sys

for _p in ("/opt/trn_rl_repo",):
    if _p not in sys.path:
        sys.path.insert(0, _p)

import math
from contextlib import ExitStack

import numpy as np

import concourse.bass as bass
import concourse.tile as tile
from concourse import bacc, mybir
from concourse.bass_utils import run_bass_kernel_spmd
from concourse.masks import make_identity

F32 = mybir.dt.float32
FP16 = mybir.dt.float16
AOP = mybir.AluOpType

B = 8192
D = 4096
H = 16
DH = 256
R = 64
DT = 0.05
NCORES = 8
BS = B // NCORES          # rows per core = 1024
P = 128                   # partitions
NT = BS // P              # row tiles per core = 8
NCH = D // P              # 128-wide column chunks per row tile = 32

FOLD_S = True             # accumulate beta*s into lin PSUM on the PE
# engine per transposed-chunk-group PSUM->SBUF copy (4 groups of 8)
HT_COPY_ENG = ("scalar", "vector", "scalar", "vector")
OUT_DVE_COLS = 2048       # cols of final out on DVE; rest on GpSimd


def _copy(nc, eng, out, in_):
    if eng == "scalar":
        nc.scalar.copy(out=out, in_=in_)
    elif eng == "vector":
        nc.vector.tensor_copy(out, in_)
    else:
        nc.gpsimd.tensor_copy(out, in_)


def _emit(tc: tile.TileContext, aps: dict, cubic_scale: float, coupling: float):
    nc = tc.nc
    beta = DT * (1.0 + coupling)
    gp = DT * coupling / (H * beta)
    gam = (beta * cubic_scale) ** (1.0 / 3.0)

    state = aps["state"]
    signal = aps["signal"]
    U_d = aps["U"]
    V_d = aps["V"]
    diag_d = aps["diag"]
    out_d = aps["out"]

    with ExitStack() as ctx:
        consts = ctx.enter_context(tc.tile_pool(name="consts", bufs=1))

        ident = consts.tile([P, P], F32, tag="ident")
        make_identity(nc, ident)
        ident16 = consts.tile([P, P], FP16, tag="ident16")
        make_identity(nc, ident16)
        identb = consts.tile([P, P], FP16, tag="identb")
        make_identity(nc, identb)
        nc.vector.tensor_scalar(
            out=identb, in0=identb, scalar1=beta, scalar2=None, op0=AOP.mult
        )

        dmasks = []
        for k in range(2):
            dmask = consts.tile([P, DH], F32, tag=f"dmask{k}")
            nc.gpsimd.memset(dmask, 0.0)
            nc.gpsimd.affine_select(
                out=dmask, in_=dmask,
                compare_op=AOP.not_equal, fill=1.0,
                base=-(k * P), pattern=[[1, DH]], channel_multiplier=-1,
            )
            dmasks.append(dmask)

        ones = consts.tile([P, 1], FP16, tag="ones")
        nc.gpsimd.memset(ones, 1.0 / DH)

        AT = consts.tile([P, H, 2, DH], FP16, tag="AT")

        # --- one-time A setup (f32 math, cast to fp16 at the end) ---
        with (
            tc.tile_pool(name="setup", bufs=2) as setup,
            tc.tile_pool(name="setup_ps", bufs=2, space="PSUM") as setup_ps,
        ):
            for h in range(H):
                u_s = setup.tile([P, 2, R], F32, tag="u_s")
                nc.sync.dma_start(out=u_s, in_=U_d[h].rearrange("(k p) r -> p k r", p=P))
                v_s = setup.tile([R, DH], F32, tag="v_s")
                nc.sync.dma_start(out=v_s, in_=V_d[h])
                dcol = setup.tile([P, 2], F32, tag="dcol")
                nc.sync.dma_start(
                    out=dcol, in_=diag_d[h].rearrange("(k p) -> p k", p=P)
                )

                ut_s = setup.tile([R, DH], F32, tag="ut_s")
                for k in range(2):
                    ut_ps = setup_ps.tile([R, P], F32, tag="ut_ps")
                    nc.tensor.transpose(ut_ps, u_s[:, k, :], ident)
                    nc.scalar.copy(out=ut_s[:, k * P:(k + 1) * P], in_=ut_ps)

                for k in range(2):
                    a_ps = setup_ps.tile([P, DH], F32, tag="a_ps")
                    nc.tensor.matmul(
                        a_ps, lhsT=v_s[:, k * P:(k + 1) * P], rhs=ut_s,
                        start=True, stop=True,
                    )
                    dg = setup.tile([P, DH], F32, tag="dg")
                    nc.vector.tensor_scalar(
                        out=dg, in0=dmasks[k],
                        scalar1=dcol[:, k:k + 1], scalar2=beta,
                        op0=AOP.mult, op1=AOP.mult,
                    )
                    nc.vector.scalar_tensor_tensor(
                        out=AT[:, h, k, :], in0=a_ps, scalar=beta, in1=dg,
                        op0=AOP.mult, op1=AOP.add,
                    )

        # --- main loop pools ---
        xp = ctx.enter_context(tc.tile_pool(name="xp", bufs=4))
        sp = ctx.enter_context(tc.tile_pool(name="sp", bufs=2))
        hp = ctx.enter_context(tc.tile_pool(name="hp", bufs=2))
        cp_ = ctx.enter_context(tc.tile_pool(name="cp", bufs=2))
        dp = ctx.enter_context(tc.tile_pool(name="dp", bufs=2))
        trp = ctx.enter_context(tc.tile_pool(name="trp", bufs=2))
        op_ = ctx.enter_context(tc.tile_pool(name="op", bufs=2))
        ps_tp = ctx.enter_context(tc.tile_pool(name="ps_tp", bufs=2, space="PSUM"))
        ps_lin = ctx.enter_context(tc.tile_pool(name="ps_lin", bufs=4, space="PSUM"))
        ps_m = ctx.enter_context(tc.tile_pool(name="ps_m", bufs=2, space="PSUM"))

        def finish_tile(it, x_t, dr_t, c3_t):
            r0 = it * P
            # drift = (lin + beta*s) + c3 full-tile join, then head-sum tree
            nc.vector.tensor_tensor(out=dr_t, in0=dr_t, in1=c3_t, op=AOP.add)
            t8 = trp.tile([P, D // 2], FP16, tag="t8", name="t8")
            nc.vector.tensor_tensor(
                t8, in0=dr_t[:, 0:D // 2], in1=dr_t[:, D // 2:D], op=AOP.add
            )
            t4 = trp.tile([P, D // 4], FP16, tag="t4", name="t4")
            nc.gpsimd.tensor_tensor(
                t4, in0=t8[:, 0:D // 4], in1=t8[:, D // 4:D // 2], op=AOP.add
            )
            t2r = trp.tile([P, D // 8], FP16, tag="t2r", name="t2r")
            nc.vector.tensor_tensor(
                t2r, in0=t4[:, 0:D // 8], in1=t4[:, D // 8:D // 4], op=AOP.add
            )
            mh2 = trp.tile([P, 2 * DH], FP16, tag="mh2", name="mh2")
            nc.vector.tensor_tensor(
                mh2[:, 0:DH], in0=t2r[:, 0:DH], in1=t2r[:, DH:2 * DH], op=AOP.add
            )
            nc.gpsimd.tensor_scalar(
                out=mh2[:, 0:DH], in0=mh2[:, 0:DH], scalar1=-gp, scalar2=None,
                op0=AOP.mult,
            )
            nc.gpsimd.tensor_copy(mh2[:, DH:2 * DH], mh2[:, 0:DH])

            # dd = drift + mh2, one 2x op per pair (a stride-0 broadcast
            # AP would drop the DVE 16-bit 2x mode)
            dd_t = cp_.tile([P, D], FP16, tag="dd", name="dd_t")
            for hp2 in range(H // 2):
                sl = slice(hp2 * 2 * DH, (hp2 + 1) * 2 * DH)
                nc.vector.tensor_tensor(
                    dd_t[:, sl], in0=dr_t[:, sl], in1=mh2, op=AOP.add,
                )
            # out = x + dd, split DVE / GpSimd
            o_t = op_.tile([P, D], FP16, tag="o", name="o_t")
            nco = OUT_DVE_COLS
            if nco > 0:
                nc.vector.tensor_tensor(
                    o_t[:, 0:nco], in0=x_t[:, 0:nco], in1=dd_t[:, 0:nco],
                    op=AOP.add,
                )
            if nco < D:
                nc.gpsimd.tensor_tensor(
                    o_t[:, nco:D], in0=x_t[:, nco:D], in1=dd_t[:, nco:D],
                    op=AOP.add,
                )
            # outputs go on the gpsimd DGE queue so the sync and scalar
            # queues never block the next tiles' input loads
            nc.gpsimd.dma_start(out=out_d[r0:r0 + P, :], in_=o_t)

        for it in range(NT):
            r0 = it * P
            x_t = xp.tile([P, D], FP16, tag="x", name="x_t")
            nc.sync.dma_start(out=x_t, in_=state[r0:r0 + P, :])
            s_t = sp.tile([P, D], FP16, tag="s", name="s_t")
            nc.scalar.dma_start(out=s_t, in_=signal[r0:r0 + P, :])

            # transposes for this tile (PE) — emitted before the previous
            # tile's tail folds so the PE always has queued work
            hT = hp.tile([P, NCH, P], FP16, tag="hT", name="hT")
            for g in range(NCH // 8):
                tp_ps = ps_tp.tile([P, 8 * P], FP16, tag="tp_ps", name="tp_ps")
                for c8 in range(8):
                    j = g * 8 + c8
                    nc.tensor.transpose(
                        tp_ps[:, c8 * P:(c8 + 1) * P],
                        x_t[:, j * P:(j + 1) * P], ident16,
                    )
                _copy(
                    nc, HT_COPY_ENG[g],
                    hT[:, g * 8:(g + 1) * 8, :].rearrange("p a b -> p (a b)"),
                    tp_ps,
                )

            m_ps = ps_m.tile([P, H], F32, tag="m_ps", name="m_ps")
            ct_t = cp_.tile([P, D], FP16, tag="ct", name="ct_t")
            c2_t = cp_.tile([P, D], FP16, tag="c2", name="c2_t")
            c3_t = cp_.tile([P, D], FP16, tag="c3", name="c3_t")
            dr_t = dp.tile([P, D], FP16, tag="dr", name="dr_t")

            for hp2 in range(H // 2):
                l_ps = ps_lin.tile([P, 2 * DH], F32, tag="l_ps", name="l_ps")
                for hh in range(2):
                    h = hp2 * 2 + hh
                    # v1-proven PSUM pattern: each 256-wide region's
                    # accumulation group opens, accumulates, and CLOSES
                    # before the next region in the same bank opens.
                    for k in range(2):
                        j = 2 * h + k
                        nc.tensor.matmul(
                            l_ps[:, hh * DH:(hh + 1) * DH],
                            lhsT=hT[:, j, :], rhs=AT[:, h, k, :],
                            start=(k == 0),
                            stop=(k == 1) and not FOLD_S,
                        )
                        nc.tensor.matmul(
                            m_ps[:, h:h + 1],
                            lhsT=hT[:, j, :], rhs=ones,
                            start=(k == 0), stop=(k == 1),
                        )
                    if FOLD_S:
                        nc.tensor.matmul(
                            l_ps[:, hh * DH:(hh + 1) * DH],
                            lhsT=identb, rhs=s_t[:, h * DH:(h + 1) * DH],
                            start=False, stop=True,
                            skip_group_check=True,
                        )
                # lin + beta*s (pair) PSUM -> SBUF fp16, right away
                nc.scalar.copy(
                    out=dr_t[:, hp2 * 2 * DH:(hp2 + 1) * 2 * DH], in_=l_ps
                )

                # ct per head, mean read straight from PSUM
                for hh in range(2):
                    h = hp2 * 2 + hh
                    nc.vector.tensor_scalar(
                        out=ct_t[:, h * DH:(h + 1) * DH],
                        in0=x_t[:, h * DH:(h + 1) * DH],
                        scalar1=m_ps[:, h:h + 1], scalar2=gam,
                        op0=AOP.subtract, op1=AOP.mult,
                    )
                if hp2 % 2 == 1:
                    # quarter c2 (ACT Square) + c3 (DVE)
                    q = hp2 // 2
                    qs = slice(q * (D // 4), (q + 1) * (D // 4))
                    nc.scalar.activation(
                        out=c2_t[:, qs], in_=ct_t[:, qs],
                        func=mybir.ActivationFunctionType.Square,
                        scale=1.0,
                    )
                    nc.vector.tensor_tensor(
                        out=c3_t[:, qs], in0=ct_t[:, qs], in1=c2_t[:, qs],
                        op=AOP.mult,
                    )

            finish_tile(it, x_t, dr_t, c3_t)


_CACHE: dict = {}


def _build(cubic_scale: float, coupling: float) -> bass.Bass:
    key = (float(cubic_scale), float(coupling))
    if key in _CACHE:
        return _CACHE[key]
    nc = bacc.Bacc("TRN2", target_bir_lowering=False, debug=False)
    aps = {
        "state": nc.dram_tensor("state", [BS, D], FP16, kind="ExternalInput").ap(),
        "signal": nc.dram_tensor("signal", [BS, D], FP16, kind="ExternalInput").ap(),
        "U": nc.dram_tensor("U", [H, DH, R], F32, kind="ExternalInput").ap(),
        "V": nc.dram_tensor("V", [H, R, DH], F32, kind="ExternalInput").ap(),
        "diag": nc.dram_tensor("diag", [H, DH], F32, kind="ExternalInput").ap(),
        "out": nc.dram_tensor("out", [BS, D], FP16, kind="ExternalOutput").ap(),
    }
    with tile.TileContext(nc) as tc:
        _emit(tc, aps, float(cubic_scale), float(coupling))
    nc.compile()
    _CACHE[key] = nc
    return nc


def run(state, signal, U, V, diag, cubic_scale, coupling, trace=False):
    state = np.ascontiguousarray(np.asarray(state, dtype=np.float16))
    signal = np.ascontiguousarray(np.asarray(signal, dtype=np.float16))
    U = np.ascontiguousarray(np.asarray(U, dtype=np.float32))
    V = np.ascontiguousarray(np.asarray(V, dtype=np.float32))
    diag = np.ascontiguousarray(np.asarray(diag, dtype=np.float32))

    nc = _build(float(cubic_scale), float(coupling))
    in_maps = []
    for i in range(NCORES):
        sl = slice(i * BS, (i + 1) * BS)
        in_maps.append({
            "state": state[sl], "signal": signal[sl],
            "U": U, "V": V, "diag": diag,
        })
    res = run_bass_kernel_spmd(nc, in_maps, list(range(NCORES)), trace=trace)
    out = np.concatenate(
        [res.results[i]["out"] for i in range(NCORES)], axis=0
    ).astype(np.float32)
    return out, res


def kernel(state, signal, U, V, diag, cubic_scale, coupling) -> np.ndarray:
    out, _ = run(state, signal, U, V, diag, cubic_scale, coupling, trace=False)
    return out
